# revision 11
# baseline (speedup 1.0000x reference)
"""Trainium2 Bass kernel for a fused GRU cell.

Reference computation (per row b of a batch):
    z = sigmoid(x @ Wz + h @ Uz + bz)
    r = sigmoid(x @ Wr + h @ Ur + br)
    h_hat = tanh(x @ Wh + (r * h) @ Uh + bh)
    out = z * h + (1 - z) * h_hat

Shapes: x, h_prev [65536, 256] f32; six weights [256, 256]; three biases [256].

Strategy: data-parallel over the batch across 8 NeuronCores (8192 rows each).
All compute happens in a transposed [feature, row] layout so the tiny weight
matrices are the stationary matmul operands and the per-feature biases land on
the partition axis (free via the ACT bias port). The transposition and the
f32->bf16 cast of x/h (and the inverse for the output) are done on the HOST
during shard/gather — the device sees bf16 [256, rows] tensors directly, so
the kernel body is nothing but dense bf16 matmuls (PSUM fp32 accumulate),
sigmoid/tanh on ACT, and the gate blend on DVE. No on-device transposes, no
PSUM-evacuation copies beyond the activations, and 2/3 less HBM traffic than
the f32 natural layout.
"""

import numpy as np
import ml_dtypes

import concourse.mybir as mybir
import concourse.tile as tile
from concourse import bacc
from concourse.alu_op_type import AluOpType
from concourse.bass_utils import run_bass_kernel_spmd

N_CORES = 8
B, D, U = 65536, 256, 256
ROWS_PER_CORE = B // N_CORES  # 8192
CHUNK = 512        # rows per compute pipeline iteration
LOAD_ROWS = 512    # rows per DMA load transfer

F32 = mybir.dt.float32
BF16 = mybir.dt.bfloat16
AF = mybir.ActivationFunctionType
BFNP = ml_dtypes.bfloat16

W_NAMES = ["Wz", "Uz", "Wr", "Ur", "Wh", "Uh"]
B_NAMES = ["bz", "br", "bh"]


def build_module(rows_per_core: int = ROWS_PER_CORE, iters: int = 1):
    """Emit + compile the per-core module. `iters` repeats the whole body
    (hardware loop) for wall-clock timing; the output is idempotent."""
    assert rows_per_core % LOAD_ROWS == 0 and LOAD_ROWS % CHUNK == 0
    nload = rows_per_core // LOAD_ROWS
    sub_per_load = LOAD_ROWS // CHUNK
    nchunk = rows_per_core // CHUNK

    nc = bacc.Bacc("TRN2", target_bir_lowering=False, debug=False)

    # Device-side tensors: x/h arrive pre-transposed bf16 [feat, rows];
    # the output leaves transposed bf16 as well.
    xT_d = nc.dram_tensor("xT", [D, rows_per_core], BF16, kind="ExternalInput").ap()
    hT_d = nc.dram_tensor("hT", [U, rows_per_core], BF16, kind="ExternalInput").ap()
    w_d = {n: nc.dram_tensor(n, [D, U], BF16, kind="ExternalInput").ap() for n in W_NAMES}
    b_d = {n: nc.dram_tensor(n, [U], F32, kind="ExternalInput").ap() for n in B_NAMES}
    outT_d = nc.dram_tensor("outT", [U, rows_per_core], BF16, kind="ExternalOutput").ap()

    with tile.TileContext(nc) as tc:
        with (
            tc.tile_pool(name="consts", bufs=1) as cpool,
            tc.tile_pool(name="loads", bufs=6) as lpool,
            tc.tile_pool(name="work", bufs=3) as wpool,
            tc.tile_pool(name="psum_mm", bufs=8, space="PSUM") as pmm,
        ):
            # ---- constants (loaded once; on the ACT HWDGE ring so the
            # first x/h loads on the sync ring are not queued behind them) ----
            w_s = {}
            for n in W_NAMES:
                wt = cpool.tile([128, 2, U], BF16, name=f"ws_{n}")
                nc.scalar.dma_start(wt[:], w_d[n].rearrange("(kk p) n -> p kk n", p=128))
                w_s[n] = wt
            b_s = {}
            for n in B_NAMES:
                bt = cpool.tile([128, 2], F32, name=f"bs_{n}")
                nc.scalar.dma_start(bt[:], b_d[n].rearrange("(u p) -> p u", p=128))
                b_s[n] = bt

            def mm(out, w_name, kk, u, rhs, start, stop):
                lhsT = w_s[w_name][:, kk, u * 128:(u + 1) * 128]
                nc.tensor.matmul(out, lhsT, rhs, start=start, stop=stop)

            x_tiles = [None] * nload
            h_tiles = [None] * nload
            stash = {}

            def emit_load(l):
                xt = lpool.tile([128, 2, LOAD_ROWS], BF16, name="xL")
                ht = lpool.tile([128, 2, LOAD_ROWS], BF16, name="hL")
                sl = slice(l * LOAD_ROWS, (l + 1) * LOAD_ROWS)
                nc.sync.dma_start(
                    xt[:], xT_d[:, sl].rearrange("(kk p) r -> p kk r", p=128))
                nc.sync.dma_start(
                    ht[:], hT_d[:, sl].rearrange("(kk p) r -> p kk r", p=128))
                x_tiles[l] = xt
                h_tiles[l] = ht

            def emit_front(c):
                l, sub = divmod(c, sub_per_load)
                rsl = slice(sub * CHUNK, (sub + 1) * CHUNK)
                xT = x_tiles[l][:, :, rsl]
                hT = h_tiles[l][:, :, rsl]

                # ---- gate pre-activations: r, z (PSUM fp32) ----
                ps_r = [pmm.tile([128, CHUNK], F32, name="ps_r", tag="gates") for _ in range(2)]
                ps_z = [pmm.tile([128, CHUNK], F32, name="ps_z", tag="gates") for _ in range(2)]
                for u in range(2):
                    mm(ps_r[u][:], "Wr", 0, u, xT[:, 0, :], True, False)
                    mm(ps_r[u][:], "Wr", 1, u, xT[:, 1, :], False, False)
                    mm(ps_r[u][:], "Ur", 0, u, hT[:, 0, :], False, False)
                    mm(ps_r[u][:], "Ur", 1, u, hT[:, 1, :], False, True)
                for u in range(2):
                    mm(ps_z[u][:], "Wz", 0, u, xT[:, 0, :], True, False)
                    mm(ps_z[u][:], "Wz", 1, u, xT[:, 1, :], False, False)
                    mm(ps_z[u][:], "Uz", 0, u, hT[:, 0, :], False, False)
                    mm(ps_z[u][:], "Uz", 1, u, hT[:, 1, :], False, True)

                r_s = wpool.tile([128, 2, CHUNK], BF16, name="r_s")
                z_s = wpool.tile([128, 2, CHUNK], BF16, name="z_s")
                for u in range(2):
                    nc.scalar.activation(r_s[:, u, :], ps_r[u][:], AF.Sigmoid,
                                         bias=b_s["br"][:, u:u + 1])
                for u in range(2):
                    nc.scalar.activation(z_s[:, u, :], ps_z[u][:], AF.Sigmoid,
                                         bias=b_s["bz"][:, u:u + 1])

                # ---- rh = r * h (transposed layout) ----
                rh = wpool.tile([128, 2, CHUNK], BF16, name="rh")
                nc.vector.tensor_tensor(rh[:], r_s[:], hT, AluOpType.mult)
                stash[c] = (z_s, rh)

            def emit_back(c):
                l, sub = divmod(c, sub_per_load)
                rsl = slice(sub * CHUNK, (sub + 1) * CHUNK)
                xT = x_tiles[l][:, :, rsl]
                hT = h_tiles[l][:, :, rsl]
                z_s, rh = stash.pop(c)

                # ---- h_hat pre-activation: x @ Wh + rh @ Uh ----
                ps_g = [pmm.tile([128, CHUNK], F32, name="ps_g", tag="gates") for _ in range(2)]
                for u in range(2):
                    mm(ps_g[u][:], "Wh", 0, u, xT[:, 0, :], True, False)
                    mm(ps_g[u][:], "Wh", 1, u, xT[:, 1, :], False, False)
                    mm(ps_g[u][:], "Uh", 0, u, rh[:, 0, :], False, False)
                    mm(ps_g[u][:], "Uh", 1, u, rh[:, 1, :], False, True)

                hh = wpool.tile([128, 2, CHUNK], BF16, name="hh")
                dlt = wpool.tile([128, 2, CHUNK], BF16, name="dlt")
                ho = wpool.tile([128, 2, CHUNK], BF16, name="ho")

                # The final chunk's epilogue is split in half so the very
                # last store waits on a half-size tanh/blend chain.
                halves = 2 if c == nchunk - 1 else 1
                hrows = CHUNK // halves
                for hv in range(halves):
                    hsl = slice(hv * hrows, (hv + 1) * hrows)
                    for u in range(2):
                        nc.scalar.activation(hh[:, u, hsl], ps_g[u][:, hsl],
                                             AF.Tanh, bias=b_s["bh"][:, u:u + 1])

                    # ---- blend: out = hh + z * (h - hh) ----
                    nc.vector.tensor_tensor(dlt[:, :, hsl], hT[:, :, hsl],
                                            hh[:, :, hsl], AluOpType.subtract)
                    nc.vector.tensor_tensor(dlt[:, :, hsl], z_s[:, :, hsl],
                                            dlt[:, :, hsl], AluOpType.mult)
                    nc.vector.tensor_tensor(ho[:, :, hsl], hh[:, :, hsl],
                                            dlt[:, :, hsl], AluOpType.add)

                    sl = slice(c * CHUNK + hv * hrows,
                               c * CHUNK + (hv + 1) * hrows)
                    nc.gpsimd.dma_start(
                        outT_d[:, sl].rearrange("(kk p) r -> p kk r", p=128),
                        ho[:, :, hsl])

            def emit_all():
                prefetch = 2 * sub_per_load
                for l in range(min(2, nload)):
                    emit_load(l)
                for c in range(nchunk):
                    l, sub = divmod(c + prefetch, sub_per_load)
                    if sub == 0 and l < nload:
                        emit_load(l)
                    emit_front(c)
                    if c > 0:
                        emit_back(c - 1)
                emit_back(nchunk - 1)

            if iters == 1:
                emit_all()
            else:
                with tc.For_i(0, iters, 1, hint_engines=(
                        mybir.EngineType.PE,
                        mybir.EngineType.Activation,
                        mybir.EngineType.DVE)):
                    emit_all()

    nc.compile()
    return nc


_NC_CACHE: dict = {}


def _get_module(rows_per_core: int = ROWS_PER_CORE, iters: int = 1):
    key = (rows_per_core, iters)
    if key not in _NC_CACHE:
        _NC_CACHE[key] = build_module(rows_per_core, iters)
    return _NC_CACHE[key]


def make_in_maps(inputs: dict) -> list:
    """Host-side shard: per-core transposed bf16 x/h + replicated weights."""
    x = np.asarray(inputs["x"], dtype=np.float32)
    h = np.asarray(inputs["h_prev"], dtype=np.float32)
    assert x.shape == (B, D) and h.shape == (B, U)
    # [B, D] -> [cores, D, rows] bf16, contiguous
    xT = np.ascontiguousarray(
        x.astype(BFNP).reshape(N_CORES, ROWS_PER_CORE, D).transpose(0, 2, 1))
    hT = np.ascontiguousarray(
        h.astype(BFNP).reshape(N_CORES, ROWS_PER_CORE, U).transpose(0, 2, 1))
    consts = {n: np.ascontiguousarray(np.asarray(inputs[n]).astype(BFNP))
              for n in W_NAMES}
    consts.update({n: np.ascontiguousarray(np.asarray(inputs[n], dtype=np.float32))
                   for n in B_NAMES})
    return [{"xT": xT[c], "hT": hT[c], **consts} for c in range(N_CORES)]


def gather_out(results) -> np.ndarray:
    """Host-side gather: un-transpose, widen to f32, concat over cores."""
    outs = [np.asarray(results[c]["outT"]).astype(np.float32).T
            for c in range(N_CORES)]
    return np.ascontiguousarray(np.concatenate(outs, axis=0))


def kernel(**inputs: np.ndarray) -> np.ndarray:
    nc = _get_module()
    in_maps = make_in_maps(inputs)
    res = run_bass_kernel_spmd(nc, in_maps, core_ids=list(range(N_CORES)))
    return gather_out(res.results)
